# revision 13
# baseline (speedup 1.0000x reference)
"""Distributed causal attention kernel for Trainium2 (8 NeuronCores).

Tensor-parallel over heads: core c owns heads {2c, 2c+1}.

v2 design:
- x is transposed HOST-side and uploaded as xT [D, S] bf16 (no on-chip
  transpose of x).
- Per window w (512 seq cols): QKV projection from resident xT slice,
  RoPE (host-permuted Wproj rows make it contiguous elementwise),
  causal attention in keys-on-partitions layout, exp split between the
  Activation engine (exact) and DVE (Schraudolph bf16-bits exp via
  tensor_scalar -> int16 bitcast), causal mask multiplies on DVE/Pool.
- Attention output is sent UNNORMALIZED (plus per-query rowsums) via one
  AllGather per window (overlapped with the next window's compute).
- At the end each core gathers ITS 512-row block from the AllGather
  outputs with dma_gather (rank-dependent int16 indices are host input),
  normalizes (reciprocal + selector-matmul broadcast), applies the
  output projection, and writes its [512, 1024] block. Host concatenates.
"""
import sys

if "/opt/trn_rl_repo" not in sys.path:
    sys.path.insert(0, "/opt/trn_rl_repo")

import math
from contextlib import ExitStack

import numpy as np
import ml_dtypes

import concourse.bass as bass
import concourse.bacc as bacc
import concourse.tile as tile
import concourse.mybir as mybir
from concourse.bass_utils import run_bass_kernel_spmd
from concourse.masks import make_identity

BF16 = mybir.dt.bfloat16
F32 = mybir.dt.float32
I16 = mybir.dt.int16

N_CORES = 8
S = 4096
D = 1024
H = 16
DH = 64
HPC = H // N_CORES          # heads per core = 2
W = 512                     # seq window
NW = S // W                 # windows
ROWS = S // N_CORES         # output rows per core = 512

# Schraudolph bf16-bits exp: exp(s/8) ~ bf16_bits(round(EXPA*s + EXPB))
EXPA = 128.0 / (8.0 * math.log(2.0))
EXPB = 127.0 * 128.0 - 7.4

GBLK = 256                  # padded AllGather block rows per core


def build_graph():
    nc = bacc.Bacc("TRN2", target_bir_lowering=False, debug=False,
                   num_devices=N_CORES)

    xt_in = nc.dram_tensor("xt", [D, S], BF16, kind="ExternalInput").ap()
    wt_in = nc.dram_tensor("wt", [D, 3 * HPC * DH], BF16, kind="ExternalInput").ap()
    wot_in = nc.dram_tensor("wot", [D, D], BF16, kind="ExternalInput").ap()
    cos_in = nc.dram_tensor("cos", [128, S], BF16, kind="ExternalInput").ap()
    sin_in = nc.dram_tensor("sin", [128, S], BF16, kind="ExternalInput").ap()
    msk_in = nc.dram_tensor("msk", [128, 4 * 2 * W], BF16, kind="ExternalInput").ap()
    esel_in = nc.dram_tensor("esel", [16, 8 * 128], BF16, kind="ExternalInput").ap()
    gidx_in = nc.dram_tensor("gidx", [128, 64], I16, kind="ExternalInput").ap()
    sidx_in = nc.dram_tensor("sidx", [128, 1], I16, kind="ExternalInput").ap()
    out_ext = nc.dram_tensor("out", [ROWS, D], F32, kind="ExternalOutput").ap()

    with tile.TileContext(nc) as tc:
        _kernel_body(tc, nc, xt_in, wt_in, wot_in, cos_in, sin_in, msk_in,
                     esel_in, gidx_in, sidx_in, out_ext)

    nc.compile()
    return nc


def _kernel_body(tc, nc, xt_in, wt_in, wot_in, cos_in, sin_in, msk_in,
                 esel_in, gidx_in, sidx_in, out_ext):
    ctx = ExitStack()

    consts = ctx.enter_context(tc.tile_pool(name="consts", bufs=1))
    xtp = ctx.enter_context(tc.tile_pool(name="xtp", bufs=2))
    ropep = ctx.enter_context(tc.tile_pool(name="ropep", bufs=2))
    ptp = ctx.enter_context(tc.tile_pool(name="ptp", bufs=4))
    ysp = ctx.enter_context(tc.tile_pool(name="ysp", bufs=2))
    tailp = ctx.enter_context(tc.tile_pool(name="tailp", bufs=2))
    dram = ctx.enter_context(tc.tile_pool(name="dram", bufs=1, space="DRAM"))
    # PSUM budget: proj 2 + stp 2x2 + av 2 = 8 banks
    projp = ctx.enter_context(tc.tile_pool(name="projp", bufs=2, space="PSUM"))
    pst = ctx.enter_context(tc.tile_pool(name="pst", bufs=2, space="PSUM"))
    pav = ctx.enter_context(tc.tile_pool(name="pav", bufs=1, space="PSUM"))

    # ---- constants / weights resident in SBUF ----
    ident = consts.tile([128, 128], BF16)
    make_identity(nc, ident)

    wt_sb = consts.tile([128, 8, 3 * HPC * DH], BF16)
    nc.sync.dma_start(wt_sb[:], wt_in.rearrange("(o p) m -> p o m", p=128))
    wot_sb = consts.tile([128, 8, D], BF16)
    nc.sync.dma_start(wot_sb[:], wot_in.rearrange("(o p) m -> p o m", p=128))
    cos_sb = consts.tile([128, S], BF16)
    nc.sync.dma_start(cos_sb[:], cos_in[:])
    sin_sb = consts.tile([128, S], BF16)
    nc.sync.dma_start(sin_sb[:], sin_in[:])
    msk_sb = consts.tile([128, 4, 2, W], BF16)
    nc.sync.dma_start(msk_sb[:], msk_in.rearrange("p (a b f) -> p a b f", a=4, b=2))
    esel_sb = consts.tile([16, 8, 128], BF16)
    nc.sync.dma_start(esel_sb[:], esel_in.rearrange("p (c m) -> p c m", c=8))
    gidx_sb = consts.tile([128, 64], I16)
    nc.sync.dma_start(gidx_sb[:], gidx_in[:])
    sidx_sb = consts.tile([128, 1], I16)
    nc.sync.dma_start(sidx_sb[:], sidx_in[:])

    # persistent per-core activations
    qT = consts.tile([128, S], BF16)       # [2 heads x 64dh (e|o perm), S]
    kT = consts.tile([128, S], BF16)
    vex = consts.tile([128, S // 128, 130], BF16)  # [s, chunk, v_h0 |1| v_h1 |1]
    nc.vector.memset(vex[:, :, 64:65], 1.0)
    nc.vector.memset(vex[:, :, 129:130], 1.0)

    # AllGather staging: per-window contribution [GBLK, W] bf16
    ag_in = [dram.tile([GBLK, W], BF16, name=f"ag_in{w}") for w in range(NW)]
    # concatenated output so the final gather can index by (window, src)
    ag_out = dram.tile([NW, N_CORES * GBLK, W], BF16)
    groups = [list(range(N_CORES))]

    for w in range(NW):
        ws = slice(w * W, (w + 1) * W)
        # ---- load xT window slice (host-transposed) ----
        xt_sb = xtp.tile([128, 8, W], BF16, name="xt_sb")
        nc.sync.dma_start(
            xt_sb[:], xt_in[:, ws].rearrange("(o p) m -> p o m", p=128))

        # ---- QKV projection ----
        ev_ps = projp.tile([128, W], F32, name="ev_ps", tag="pj")
        od_ps = projp.tile([128, W], F32, name="od_ps", tag="pj")
        for dst, t in ((ev_ps, 0), (od_ps, 1)):
            for d in range(8):
                nc.tensor.matmul(dst[:], wt_sb[:, d, t * 128:(t + 1) * 128],
                                 xt_sb[:, d, :], start=(d == 0), stop=(d == 7))

        # ---- RoPE (DVE), writing qT/kT 32-row strips ----
        # order frees ev_ps after two ops so the V projection can start
        cw = cos_sb[:, ws]
        sw = sin_sb[:, ws]
        t1 = ropep.tile([128, W], BF16, name="t1")
        t2 = ropep.tile([128, W], BF16, name="t2")
        t3 = ropep.tile([128, W], BF16, name="t3")
        re = ropep.tile([128, W], BF16, name="re")
        ro = ropep.tile([128, W], BF16, name="ro")
        nc.vector.tensor_tensor(t1[:], ev_ps[:], cw, mybir.AluOpType.mult)
        nc.vector.tensor_tensor(t3[:], ev_ps[:], sw, mybir.AluOpType.mult)
        nc.vector.tensor_tensor(t2[:], od_ps[:], sw, mybir.AluOpType.mult)
        nc.vector.tensor_tensor(re[:], t1[:], t2[:], mybir.AluOpType.subtract)
        nc.vector.tensor_tensor(t2[:], od_ps[:], cw, mybir.AluOpType.mult)
        nc.vector.tensor_tensor(ro[:], t3[:], t2[:], mybir.AluOpType.add)
        for h in range(2):
            nc.gpsimd.tensor_copy(qT[h * 64:h * 64 + 32, ws],
                                  re[h * 32:(h + 1) * 32, :])
            nc.gpsimd.tensor_copy(qT[h * 64 + 32:h * 64 + 64, ws],
                                  ro[h * 32:(h + 1) * 32, :])
            nc.gpsimd.tensor_copy(kT[h * 64:h * 64 + 32, ws],
                                  re[64 + h * 32:64 + (h + 1) * 32, :])
            nc.gpsimd.tensor_copy(kT[h * 64 + 32:h * 64 + 64, ws],
                                  ro[64 + h * 32:64 + (h + 1) * 32, :])

        # ---- V: project, transpose to seq-major, pack into vex ----
        v_ps = projp.tile([128, W], F32, name="v_ps", tag="pj")
        for d in range(8):
            nc.tensor.matmul(v_ps[:], wt_sb[:, d, 256:384],
                             xt_sb[:, d, :], start=(d == 0), stop=(d == 7))
        vT = ropep.tile([128, W], BF16, name="vT")
        nc.vector.tensor_copy(vT[:], v_ps[:])
        ptv = projp.tile([128, W], BF16, name="ptv", tag="pj")
        for b in range(4):
            nc.tensor.transpose(ptv[:, b * 128:(b + 1) * 128],
                                vT[:, b * 128:(b + 1) * 128], ident)
        ptv4 = ptv.rearrange("p (b d) -> p b d", b=4)
        nc.vector.tensor_copy(vex[:, w * 4:(w + 1) * 4, 0:64], ptv4[:, :, 0:64])
        nc.vector.tensor_copy(vex[:, w * 4:(w + 1) * 4, 65:129],
                              ptv4[:, :, 64:128])

        # ---- attention for window w (keys 0 .. (w+1)*512), both heads ----
        nk = 4 * (w + 1)
        av0 = pav.tile([65, W], F32, name="av0")
        av1 = pav.tile([65, W], F32, name="av1")
        avs = (av0, av1)

        pts = [None] * nk

        def emit_qk(j):
            stp = pst.tile([128, 2, W], F32, name="stp")
            for h in range(2):
                hs = slice(h * 64, (h + 1) * 64)
                nc.tensor.matmul(stp[:, h, :], kT[hs, j * 128:(j + 1) * 128],
                                 qT[hs, ws], start=True, stop=True)
            jl = j - 4 * w
            diag = jl >= 0
            use_dve = (not diag) and (j % 2 == 0)
            if use_dve:
                pti = ptp.tile([128, 2, W], I16, name="pti")
                nc.vector.tensor_scalar(pti[:], stp[:], EXPA, EXPB,
                                        mybir.AluOpType.mult,
                                        mybir.AluOpType.add)
                pt = pti
            else:
                pt = ptp.tile([128, 2, W], BF16, name="pt")
                nc.scalar.activation(pt[:], stp[:],
                                     mybir.ActivationFunctionType.Exp,
                                     scale=1.0 / math.sqrt(DH))
            if diag:
                eng = nc.vector if (jl % 2 == 0) else nc.gpsimd
                eng.tensor_tensor(pt[:], pt[:], msk_sb[:, jl],
                                  mybir.AluOpType.mult)
            pts[j] = (pt, use_dve)

        def emit_av(j):
            pt, is_i16 = pts[j]
            for h in range(2):
                pth = pt[:, h, :]
                if is_i16:
                    pth = pth.bitcast(BF16)
                nc.tensor.matmul(avs[h][:], vex[:, j, h * 65:(h + 1) * 65],
                                 pth,
                                 start=(j == 0), stop=(j == nk - 1))
            pts[j] = None

        # software pipeline: QK_{j} runs ahead of AV_{j-1} on the PE queue
        emit_qk(0)
        for j in range(1, nk):
            emit_qk(j)
            emit_av(j - 1)
        emit_av(nk - 1)

        # ---- send unnormalized y + rowsums (Act copies, bf16) ----
        ys = ysp.tile([128, W], BF16, name="ys")
        ss = ysp.tile([64, W], BF16, name="ss")
        for h in range(2):
            nc.scalar.copy(ys[h * 64:(h + 1) * 64, :], avs[h][0:64, :])
            nc.vector.tensor_copy(ss[32 * h:32 * h + 1, :], avs[h][64:65, :])
        nc.sync.dma_start(ag_in[w][0:128, :], ys[:])
        nc.sync.dma_start(ag_in[w][128:129, :], ss[0:1, :])
        nc.sync.dma_start(ag_in[w][129:130, :], ss[32:33, :])
        nc.gpsimd.collective_compute(
            "AllGather", mybir.AluOpType.bypass, replica_groups=groups,
            ins=[ag_in[w].opt()], outs=[ag_out[w].opt()])

    # ---- tail: gather my block, normalize, output projection ----
    # y rows: idx[128*c + p] = myrank*N_CORES*GBLK + c*GBLK + p
    ag_flat = ag_out.rearrange("w b f -> (w b) f")
    yg = consts.tile([128, 8, W], BF16)
    nc.gpsimd.dma_gather(yg[:], ag_flat, gidx_sb[:], 1024, 1024, W)
    srows = consts.tile([128, 1, W], BF16)
    nc.gpsimd.dma_gather(srows[:], ag_flat, sidx_sb[:], 16, 16, W)

    rec = tailp.tile([16, W], F32, name="rec")
    nc.vector.reciprocal(rec[:], srows[0:16, 0, :])
    recb = tailp.tile([16, W], BF16, name="recb")
    nc.vector.tensor_copy(recb[:], rec[:])

    yn = consts.tile([128, 8, W], BF16)
    for c in range(8):
        rb = projp.tile([128, W], F32, name="rb", tag="pj")
        nc.tensor.matmul(rb[:], esel_sb[:, c, :], recb[:],
                         start=True, stop=True)
        nc.vector.tensor_tensor(yn[:, c, :], yg[:, c, :], rb[:],
                                mybir.AluOpType.mult)

    out_sb = tailp.tile([128, 4, D], F32, name="out_sb")
    for r in range(4):
        for nh in range(2):
            po = projp.tile([128, W], F32, name="po", tag="pj")
            for c in range(8):
                nc.tensor.matmul(po[:], yn[:, c, r * 128:(r + 1) * 128],
                                 wot_sb[:, c, nh * W:(nh + 1) * W],
                                 start=(c == 0), stop=(c == 7))
            nc.scalar.copy(out_sb[:, r, nh * W:(nh + 1) * W], po[:])
    nc.sync.dma_start(out_ext.rearrange("(a p) m -> p a m", p=128), out_sb[:])

    ctx.close()


_NC_CACHE = None


def _host_inputs(x, Wproj, Wo):
    """Host-side per-core input arrays (weight layout prep + tables)."""
    bf = ml_dtypes.bfloat16
    xt = np.ascontiguousarray(np.asarray(x).T).astype(bf)  # [D, S]

    invf = 1.0 / 10000.0 ** (np.arange(0, DH, 2) / DH)
    ang = np.outer(invf, np.arange(S))  # [32, S]
    cos_t = np.ascontiguousarray(np.tile(np.cos(ang), (4, 1))).astype(bf)
    sin_t = np.ascontiguousarray(np.tile(np.sin(ang), (4, 1))).astype(bf)

    p = np.arange(128)[:, None]
    f = np.arange(W)[None, :]
    msk = np.concatenate(
        [np.tile((128 * jl + p <= f).astype(np.float32), (1, 2))
         for jl in range(4)], axis=1).astype(bf)  # [128, 4*2*512]

    wot = np.ascontiguousarray(Wo.T).astype(bf)  # [D, D]

    # esel[i, c*128 + p] = 1 if i == 2c + (p >= 64)
    esel = np.zeros((16, 8, 128), dtype=np.float32)
    for c in range(8):
        esel[2 * c, c, 0:64] = 1.0
        esel[2 * c + 1, c, 64:128] = 1.0
    esel = np.ascontiguousarray(esel.reshape(16, 8 * 128)).astype(bf)

    in_maps = []
    for c in range(N_CORES):
        h0, h1 = 2 * c, 2 * c + 1
        Wq = [Wproj[64 * h:64 * h + 64, :] for h in (h0, h1)]
        Wk = [Wproj[1024 + 64 * h:1024 + 64 * h + 64, :] for h in (h0, h1)]
        Wv = [Wproj[2048 + 64 * h:2048 + 64 * h + 64, :] for h in (h0, h1)]
        evens = np.concatenate([Wq[0][::2], Wq[1][::2], Wk[0][::2], Wk[1][::2]], 0)
        odds = np.concatenate([Wq[0][1::2], Wq[1][1::2], Wk[0][1::2], Wk[1][1::2]], 0)
        vs = np.concatenate([Wv[0], Wv[1]], 0)
        wt = np.ascontiguousarray(
            np.concatenate([evens, odds, vs], 0).T).astype(bf)  # [1024, 384]

        # gather indices: my window block is window c; row layout of
        # ag_out flattened: (w * N_CORES + s) * GBLK + row.
        # dma_gather consumes idx i from [i % 16, i // 16] of a [128, n/16]
        # int16 SBUF tile (rows 16..127 unused but must hold valid values).
        base = c * N_CORES * GBLK
        gidx = np.empty(1024, dtype=np.int16)
        for s in range(8):
            gidx[128 * s:128 * (s + 1)] = base + s * GBLK + np.arange(128)
        sidx = np.empty(16, dtype=np.int16)
        for s in range(8):
            sidx[2 * s:2 * s + 2] = base + s * GBLK + 128 + np.arange(2)
        gidx_w = np.tile(np.ascontiguousarray(gidx.reshape(64, 16).T), (8, 1))
        sidx_w = np.tile(np.ascontiguousarray(sidx.reshape(1, 16).T), (8, 1))
        in_maps.append({
            "xt": xt, "wt": wt, "wot": wot,
            "cos": cos_t, "sin": sin_t, "msk": msk, "esel": esel,
            "gidx": np.ascontiguousarray(gidx_w),
            "sidx": np.ascontiguousarray(sidx_w),
        })
    return in_maps


def kernel(x, mask, Wproj, Wo):
    global _NC_CACHE
    if _NC_CACHE is None:
        _NC_CACHE = build_graph()
    nc = _NC_CACHE
    in_maps = _host_inputs(np.asarray(x), np.asarray(Wproj), np.asarray(Wo))
    res = run_bass_kernel_spmd(nc, in_maps, core_ids=list(range(N_CORES)))
    out = np.concatenate([res.results[c]["out"] for c in range(N_CORES)], axis=0)
    return np.ascontiguousarray(out.astype(np.float32))


# revision 20
# speedup vs baseline: 1.0424x; 1.0424x over previous
"""Distributed causal attention kernel for Trainium2 (8 NeuronCores).

Tensor-parallel over heads: core c owns heads {2c, 2c+1}.

v2 design:
- x is transposed HOST-side and uploaded as xT [D, S] bf16 (no on-chip
  transpose of x).
- Per window w (512 seq cols): QKV projection from resident xT slice,
  RoPE (host-permuted Wproj rows make it contiguous elementwise),
  causal attention in keys-on-partitions layout, exp split between the
  Activation engine (exact) and DVE (Schraudolph bf16-bits exp via
  tensor_scalar -> int16 bitcast), causal mask multiplies on DVE/Pool.
- Attention output is sent UNNORMALIZED (plus per-query rowsums) via one
  AllGather per window (overlapped with the next window's compute).
- At the end each core gathers ITS 512-row block from the AllGather
  outputs with dma_gather (rank-dependent int16 indices are host input),
  normalizes (reciprocal + selector-matmul broadcast), applies the
  output projection, and writes its [512, 1024] block. Host concatenates.
"""
import sys

if "/opt/trn_rl_repo" not in sys.path:
    sys.path.insert(0, "/opt/trn_rl_repo")

import math
from contextlib import ExitStack

import numpy as np
import ml_dtypes

import concourse.bass as bass
import concourse.bacc as bacc
import concourse.tile as tile
import concourse.mybir as mybir
from concourse.bass_utils import run_bass_kernel_spmd
from concourse.masks import make_identity

BF16 = mybir.dt.bfloat16
F32 = mybir.dt.float32
I16 = mybir.dt.int16

N_CORES = 8
S = 4096
D = 1024
H = 16
DH = 64
HPC = H // N_CORES          # heads per core = 2
W = 512                     # seq window
NW = S // W                 # windows
ROWS = S // N_CORES         # output rows per core = 512

# Schraudolph bf16-bits exp: exp(s/8) ~ bf16_bits(round(EXPA*s + EXPB))
EXPA = 128.0 / (8.0 * math.log(2.0))
EXPB = 127.0 * 128.0 - 7.4

GBLK = 130                  # AllGather block rows per core (128 y + 2 sums)


def build_graph():
    nc = bacc.Bacc("TRN2", target_bir_lowering=False, debug=False,
                   num_devices=N_CORES)

    xt_in = nc.dram_tensor("xt", [D, S], BF16, kind="ExternalInput").ap()
    wt_in = nc.dram_tensor("wt", [D, 3 * HPC * DH], BF16, kind="ExternalInput").ap()
    wot_in = nc.dram_tensor("wot", [D, D], BF16, kind="ExternalInput").ap()
    cos_in = nc.dram_tensor("cos", [128, S], BF16, kind="ExternalInput").ap()
    sin_in = nc.dram_tensor("sin", [128, S], BF16, kind="ExternalInput").ap()
    msk_in = nc.dram_tensor("msk", [128, 2 * 128], BF16, kind="ExternalInput").ap()
    esel_in = nc.dram_tensor("esel", [16, 8 * 128], BF16, kind="ExternalInput").ap()
    gidx_in = nc.dram_tensor("gidx", [128, 64], I16, kind="ExternalInput").ap()
    sidx_in = nc.dram_tensor("sidx", [128, 1], I16, kind="ExternalInput").ap()
    out_ext = nc.dram_tensor("out", [ROWS, D], F32, kind="ExternalOutput").ap()

    with tile.TileContext(nc) as tc:
        _kernel_body(tc, nc, xt_in, wt_in, wot_in, cos_in, sin_in, msk_in,
                     esel_in, gidx_in, sidx_in, out_ext)

    nc.compile()
    return nc


def _kernel_body(tc, nc, xt_in, wt_in, wot_in, cos_in, sin_in, msk_in,
                 esel_in, gidx_in, sidx_in, out_ext):
    ctx = ExitStack()

    consts = ctx.enter_context(tc.tile_pool(name="consts", bufs=1))
    xtp = ctx.enter_context(tc.tile_pool(name="xtp", bufs=2))
    ropep = ctx.enter_context(tc.tile_pool(name="ropep", bufs=2))
    ptp = ctx.enter_context(tc.tile_pool(name="ptp", bufs=4))
    ysp = ctx.enter_context(tc.tile_pool(name="ysp", bufs=2))
    tailp = ctx.enter_context(tc.tile_pool(name="tailp", bufs=2))
    dram = ctx.enter_context(tc.tile_pool(name="dram", bufs=1, space="DRAM"))
    # PSUM budget: proj 2 + stp 2x2 + av 2 = 8 banks
    projp = ctx.enter_context(tc.tile_pool(name="projp", bufs=2, space="PSUM"))
    pst = ctx.enter_context(tc.tile_pool(name="pst", bufs=2, space="PSUM"))
    pav = ctx.enter_context(tc.tile_pool(name="pav", bufs=1, space="PSUM"))

    # ---- constants / weights resident in SBUF ----
    ident = consts.tile([128, 128], BF16)
    make_identity(nc, ident)

    wt_sb = consts.tile([128, 8, 3 * HPC * DH], BF16)
    nc.sync.dma_start(wt_sb[:], wt_in.rearrange("(o p) m -> p o m", p=128))
    cos_sb = consts.tile([128, S], BF16)
    nc.sync.dma_start(cos_sb[:], cos_in[:])
    sin_sb = consts.tile([128, S], BF16)
    nc.sync.dma_start(sin_sb[:], sin_in[:])
    msk_sb = consts.tile([128, 2, 128], BF16)
    nc.sync.dma_start(msk_sb[:], msk_in.rearrange("p (b f) -> p b f", b=2))
    esel_sb = consts.tile([16, 8, 128], BF16)
    nc.sync.dma_start(esel_sb[:], esel_in.rearrange("p (c m) -> p c m", c=8))
    gidx_sb = consts.tile([128, 64], I16)
    nc.sync.dma_start(gidx_sb[:], gidx_in[:])
    sidx_sb = consts.tile([128, 1], I16)
    nc.sync.dma_start(sidx_sb[:], sidx_in[:])

    # persistent per-core activations
    qT = consts.tile([128, S], BF16)       # [2 heads x 64dh (e|o perm), S]
    kT = consts.tile([128, S], BF16)
    vex = consts.tile([128, S // 128, 130], BF16)  # [s, chunk, v_h0 |1| v_h1 |1]
    nc.vector.memset(vex[:, :, 64:65], 1.0)
    nc.vector.memset(vex[:, :, 129:130], 1.0)

    # AllGather staging: per-window contribution [GBLK, W] bf16
    ag_in = [dram.tile([GBLK, W], BF16, name=f"ag_in{w}") for w in range(NW)]
    # concatenated output so the final gather can index by (window, src)
    ag_out = dram.tile([NW, N_CORES * GBLK, W], BF16)
    groups = [list(range(N_CORES))]

    for w in range(NW):
        ws = slice(w * W, (w + 1) * W)
        # ---- load xT window slice (host-transposed) ----
        xt_sb = xtp.tile([128, 8, W], BF16, name="xt_sb")
        nc.sync.dma_start(
            xt_sb[:], xt_in[:, ws].rearrange("(o p) m -> p o m", p=128))

        # ---- QKV projection ----
        ev_ps = projp.tile([128, W], F32, name="ev_ps", tag="pj")
        od_ps = projp.tile([128, W], F32, name="od_ps", tag="pj")
        for dst, t in ((ev_ps, 0), (od_ps, 1)):
            for d in range(8):
                nc.tensor.matmul(dst[:], wt_sb[:, d, t * 128:(t + 1) * 128],
                                 xt_sb[:, d, :], start=(d == 0), stop=(d == 7))

        # ---- RoPE (DVE), writing qT/kT 32-row strips ----
        # order frees ev_ps after two ops so the V projection can start
        cw = cos_sb[:, ws]
        sw = sin_sb[:, ws]
        t1 = ropep.tile([128, W], BF16, name="t1")
        t2 = ropep.tile([128, W], BF16, name="t2")
        t3 = ropep.tile([128, W], BF16, name="t3")
        re = ropep.tile([128, W], BF16, name="re")
        ro = ropep.tile([128, W], BF16, name="ro")
        nc.vector.tensor_tensor(t1[:], ev_ps[:], cw, mybir.AluOpType.mult)
        nc.vector.tensor_tensor(t3[:], ev_ps[:], sw, mybir.AluOpType.mult)
        nc.vector.tensor_tensor(t2[:], od_ps[:], sw, mybir.AluOpType.mult)
        nc.vector.tensor_tensor(re[:], t1[:], t2[:], mybir.AluOpType.subtract)
        nc.vector.tensor_tensor(t2[:], od_ps[:], cw, mybir.AluOpType.mult)
        nc.vector.tensor_tensor(ro[:], t3[:], t2[:], mybir.AluOpType.add)
        # qT copies on DVE (they gate this window's QK); kT on Pool (only
        # the trailing diagonal chunks need this window's keys)
        for h in range(2):
            nc.vector.tensor_copy(qT[h * 64:h * 64 + 32, ws],
                                  re[h * 32:(h + 1) * 32, :])
            nc.vector.tensor_copy(qT[h * 64 + 32:h * 64 + 64, ws],
                                  ro[h * 32:(h + 1) * 32, :])
            nc.gpsimd.tensor_copy(kT[h * 64:h * 64 + 32, ws],
                                  re[64 + h * 32:64 + (h + 1) * 32, :])
            nc.gpsimd.tensor_copy(kT[h * 64 + 32:h * 64 + 64, ws],
                                  ro[64 + h * 32:64 + (h + 1) * 32, :])

        # ---- V: project, transpose to seq-major, pack into vex ----
        v_ps = projp.tile([128, W], F32, name="v_ps", tag="pj")
        for d in range(8):
            nc.tensor.matmul(v_ps[:], wt_sb[:, d, 256:384],
                             xt_sb[:, d, :], start=(d == 0), stop=(d == 7))
        vT = ropep.tile([128, W], BF16, name="vT")
        nc.vector.tensor_copy(vT[:], v_ps[:])
        ptv = projp.tile([128, W], BF16, name="ptv", tag="pj")
        for b in range(4):
            nc.tensor.transpose(ptv[:, b * 128:(b + 1) * 128],
                                vT[:, b * 128:(b + 1) * 128], ident)
        ptv4 = ptv.rearrange("p (b d) -> p b d", b=4)
        nc.vector.tensor_copy(vex[:, w * 4:(w + 1) * 4, 0:64], ptv4[:, :, 0:64])
        nc.vector.tensor_copy(vex[:, w * 4:(w + 1) * 4, 65:129],
                              ptv4[:, :, 64:128])

        # ---- attention for window w (keys 0 .. (w+1)*512), both heads ----
        nk = 4 * (w + 1)
        av0 = pav.tile([65, W], F32, name="av0")
        av1 = pav.tile([65, W], F32, name="av1")
        avs = (av0, av1)

        pts = [None] * nk

        def emit_qk(j):
            jl = j - 4 * w
            diag = jl >= 0
            # diagonal chunk jl: columns < 128*jl are fully masked - skip
            c0 = 128 * jl if diag else 0
            stp = pst.tile([128, 2, W], F32, name="stp")
            for h in range(2):
                hs = slice(h * 64, (h + 1) * 64)
                nc.tensor.matmul(stp[:, h, c0:], kT[hs, j * 128:(j + 1) * 128],
                                 qT[hs, w * W + c0:(w + 1) * W],
                                 start=True, stop=True)
            use_dve = (not diag) and (j % 2 == 0)
            if use_dve:
                pti = ptp.tile([128, 2, W], I16, name="pti")
                nc.vector.tensor_scalar(pti[:], stp[:], EXPA, EXPB,
                                        mybir.AluOpType.mult,
                                        mybir.AluOpType.add)
                pt = pti
            else:
                pt = ptp.tile([128, 2, W], BF16, name="pt")
                nc.scalar.activation(pt[:, :, c0:], stp[:, :, c0:],
                                     mybir.ActivationFunctionType.Exp,
                                     scale=1.0 / math.sqrt(DH))
            if diag:
                # only the 128-col block on the diagonal needs the triangle
                nc.vector.tensor_tensor(pt[:, :, c0:c0 + 128],
                                        pt[:, :, c0:c0 + 128], msk_sb[:],
                                        mybir.AluOpType.mult)
            pts[j] = (pt, use_dve, c0)

        def emit_av(j):
            pt, is_i16, c0 = pts[j]
            for h in range(2):
                pth = pt[:, h, c0:]
                if is_i16:
                    pth = pth.bitcast(BF16)
                nc.tensor.matmul(avs[h][:, c0:], vex[:, j, h * 65:(h + 1) * 65],
                                 pth,
                                 start=(j == 0), stop=(j == nk - 1))
            pts[j] = None

        # software pipeline: QK_{j} runs ahead of AV_{j-1} on the PE queue
        emit_qk(0)
        for j in range(1, nk):
            emit_qk(j)
            emit_av(j - 1)
        emit_av(nk - 1)

        # ---- send unnormalized y + rowsums (Act copies, bf16) ----
        ys = ysp.tile([128, W], BF16, name="ys")
        ss = ysp.tile([64, W], BF16, name="ss")
        for h in range(2):
            nc.scalar.copy(ys[h * 64:(h + 1) * 64, :], avs[h][0:64, :])
            nc.vector.tensor_copy(ss[32 * h:32 * h + 1, :], avs[h][64:65, :])
        nc.sync.dma_start(ag_in[w][0:128, :], ys[:])
        nc.sync.dma_start(ag_in[w][128:129, :], ss[0:1, :])
        nc.sync.dma_start(ag_in[w][129:130, :], ss[32:33, :])
        nc.gpsimd.collective_compute(
            "AllGather", mybir.AluOpType.bypass, replica_groups=groups,
            ins=[ag_in[w].opt()], outs=[ag_out[w].opt()])

    # ---- tail: gather my block, normalize, output projection ----
    wot_sb = consts.tile([128, 8, D], BF16)
    nc.sync.dma_start(wot_sb[:], wot_in.rearrange("(o p) m -> p o m", p=128))
    # y rows: idx[128*c + p] = myrank*N_CORES*GBLK + c*GBLK + p
    ag_flat = ag_out.rearrange("w b f -> (w b) f")
    yg = consts.tile([128, 8, W], BF16)
    nc.gpsimd.dma_gather(yg[:], ag_flat, gidx_sb[:], 1024, 1024, W)
    srows = consts.tile([128, 1, W], BF16)
    nc.gpsimd.dma_gather(srows[:], ag_flat, sidx_sb[:], 16, 16, W)

    rec = tailp.tile([16, W], F32, name="rec")
    nc.vector.reciprocal(rec[:], srows[0:16, 0, :])
    recb = tailp.tile([16, W], BF16, name="recb")
    nc.vector.tensor_copy(recb[:], rec[:])

    yn = consts.tile([128, 8, W], BF16)
    for c in range(8):
        rb = projp.tile([128, W], F32, name="rb", tag="pj")
        nc.tensor.matmul(rb[:], esel_sb[:, c, :], recb[:],
                         start=True, stop=True)
        nc.vector.tensor_tensor(yn[:, c, :], yg[:, c, :], rb[:],
                                mybir.AluOpType.mult)

    out_sb = tailp.tile([128, 4, D], F32, name="out_sb")
    for r in range(4):
        for nh in range(2):
            po = projp.tile([128, W], F32, name="po", tag="pj")
            for c in range(8):
                nc.tensor.matmul(po[:], yn[:, c, r * 128:(r + 1) * 128],
                                 wot_sb[:, c, nh * W:(nh + 1) * W],
                                 start=(c == 0), stop=(c == 7))
            nc.scalar.copy(out_sb[:, r, nh * W:(nh + 1) * W], po[:])
    nc.sync.dma_start(out_ext.rearrange("(a p) m -> p a m", p=128), out_sb[:])

    ctx.close()


_NC_CACHE = None


def _host_inputs(x, Wproj, Wo):
    """Host-side per-core input arrays (weight layout prep + tables)."""
    bf = ml_dtypes.bfloat16
    xt = np.ascontiguousarray(np.asarray(x).T).astype(bf)  # [D, S]

    invf = 1.0 / 10000.0 ** (np.arange(0, DH, 2) / DH)
    ang = np.outer(invf, np.arange(S))  # [32, S]
    cos_t = np.ascontiguousarray(np.tile(np.cos(ang), (4, 1))).astype(bf)
    sin_t = np.ascontiguousarray(np.tile(np.sin(ang), (4, 1))).astype(bf)

    p = np.arange(128)[:, None]
    f = np.arange(128)[None, :]
    msk = np.tile((p <= f).astype(np.float32), (1, 2)).astype(bf)  # [128, 256]

    wot = np.ascontiguousarray(Wo.T).astype(bf)  # [D, D]

    # esel[i, c*128 + p] = 1 if i == 2c + (p >= 64)
    esel = np.zeros((16, 8, 128), dtype=np.float32)
    for c in range(8):
        esel[2 * c, c, 0:64] = 1.0
        esel[2 * c + 1, c, 64:128] = 1.0
    esel = np.ascontiguousarray(esel.reshape(16, 8 * 128)).astype(bf)

    in_maps = []
    for c in range(N_CORES):
        h0, h1 = 2 * c, 2 * c + 1
        Wq = [Wproj[64 * h:64 * h + 64, :] for h in (h0, h1)]
        Wk = [Wproj[1024 + 64 * h:1024 + 64 * h + 64, :] for h in (h0, h1)]
        Wv = [Wproj[2048 + 64 * h:2048 + 64 * h + 64, :] for h in (h0, h1)]
        evens = np.concatenate([Wq[0][::2], Wq[1][::2], Wk[0][::2], Wk[1][::2]], 0)
        odds = np.concatenate([Wq[0][1::2], Wq[1][1::2], Wk[0][1::2], Wk[1][1::2]], 0)
        vs = np.concatenate([Wv[0], Wv[1]], 0)
        wt = np.ascontiguousarray(
            np.concatenate([evens, odds, vs], 0).T).astype(bf)  # [1024, 384]

        # gather indices: my window block is window c; row layout of
        # ag_out flattened: (w * N_CORES + s) * GBLK + row.
        # dma_gather consumes idx i from [i % 16, i // 16] of a [128, n/16]
        # int16 SBUF tile (rows 16..127 unused but must hold valid values).
        base = c * N_CORES * GBLK
        gidx = np.empty(1024, dtype=np.int16)
        for s in range(8):
            gidx[128 * s:128 * (s + 1)] = base + s * GBLK + np.arange(128)
        sidx = np.empty(16, dtype=np.int16)
        for s in range(8):
            sidx[2 * s:2 * s + 2] = base + s * GBLK + 128 + np.arange(2)
        gidx_w = np.tile(np.ascontiguousarray(gidx.reshape(64, 16).T), (8, 1))
        sidx_w = np.tile(np.ascontiguousarray(sidx.reshape(1, 16).T), (8, 1))
        in_maps.append({
            "xt": xt, "wt": wt, "wot": wot,
            "cos": cos_t, "sin": sin_t, "msk": msk, "esel": esel,
            "gidx": np.ascontiguousarray(gidx_w),
            "sidx": np.ascontiguousarray(sidx_w),
        })
    return in_maps


def kernel(x, mask, Wproj, Wo):
    global _NC_CACHE
    if _NC_CACHE is None:
        _NC_CACHE = build_graph()
    nc = _NC_CACHE
    in_maps = _host_inputs(np.asarray(x), np.asarray(Wproj), np.asarray(Wo))
    res = run_bass_kernel_spmd(nc, in_maps, core_ids=list(range(N_CORES)))
    out = np.concatenate([res.results[c]["out"] for c in range(N_CORES)], axis=0)
    return np.ascontiguousarray(out.astype(np.float32))


# revision 30
# speedup vs baseline: 2.1638x; 2.0758x over previous
"""Distributed causal attention kernel for Trainium2 (8 NeuronCores).

Tensor-parallel over heads: core c owns heads {2c, 2c+1}.

v2 design:
- x is transposed HOST-side and uploaded as xT [D, S] bf16 (no on-chip
  transpose of x).
- Per window w (512 seq cols): QKV projection from resident xT slice,
  RoPE (host-permuted Wproj rows make it contiguous elementwise),
  causal attention in keys-on-partitions layout, exp split between the
  Activation engine (exact) and DVE (Schraudolph bf16-bits exp via
  tensor_scalar -> int16 bitcast), causal mask multiplies on DVE/Pool.
- Attention output is sent UNNORMALIZED (plus per-query rowsums) via one
  AllGather per window (overlapped with the next window's compute).
- At the end each core gathers ITS 512-row block from the AllGather
  outputs with dma_gather (rank-dependent int16 indices are host input),
  normalizes (reciprocal + selector-matmul broadcast), applies the
  output projection, and writes its [512, 1024] block. Host concatenates.
"""
import sys

if "/opt/trn_rl_repo" not in sys.path:
    sys.path.insert(0, "/opt/trn_rl_repo")

import math
from contextlib import ExitStack

import numpy as np
import ml_dtypes

import concourse.bass as bass
import concourse.bacc as bacc
import concourse.tile as tile
import concourse.mybir as mybir
from concourse.bass_utils import run_bass_kernel_spmd
from concourse.masks import make_identity

BF16 = mybir.dt.bfloat16
F32 = mybir.dt.float32
I16 = mybir.dt.int16

N_CORES = 8
S = 4096
D = 1024
H = 16
DH = 64
HPC = H // N_CORES          # heads per core = 2
W = 512                     # seq window
NW = S // W                 # windows
ROWS = S // N_CORES         # output rows per core = 512

# Schraudolph bf16-bits exp: exp(s/8) ~ bf16_bits(round(EXPA*s + EXPB))
EXPA = 128.0 / (8.0 * math.log(2.0))
EXPB = 127.0 * 128.0 - 7.4

GBLK = 130                  # AllGather block rows per core (128 y + 2 sums)


def build_graph():
    nc = bacc.Bacc("TRN2", target_bir_lowering=False, debug=False,
                   num_devices=N_CORES)

    xt_in = nc.dram_tensor("xt", [D, S], BF16, kind="ExternalInput").ap()
    wt_in = nc.dram_tensor("wt", [D, 3 * HPC * DH], BF16, kind="ExternalInput").ap()
    wot_in = nc.dram_tensor("wot", [D, D], BF16, kind="ExternalInput").ap()
    cos_in = nc.dram_tensor("cos", [128, S], BF16, kind="ExternalInput").ap()
    sin_in = nc.dram_tensor("sin", [128, S], BF16, kind="ExternalInput").ap()
    msk_in = nc.dram_tensor("msk", [128, 2 * 128], BF16, kind="ExternalInput").ap()
    esel_in = nc.dram_tensor("esel", [16, 8 * 128], BF16, kind="ExternalInput").ap()
    goff_in = nc.dram_tensor("goff", [1, 1], mybir.dt.int32,
                             kind="ExternalInput").ap()
    ago_t = nc.dram_tensor("ag_out", [NW * N_CORES * GBLK, W], BF16,
                           kind="Internal").ap()
    out_ext = nc.dram_tensor("out", [ROWS, D], F32, kind="ExternalOutput").ap()

    with tile.TileContext(nc) as tc:
        _kernel_body(tc, nc, xt_in, wt_in, wot_in, cos_in, sin_in, msk_in,
                     esel_in, goff_in, ago_t, out_ext)

    nc.compile()
    return nc


def _kernel_body(tc, nc, xt_in, wt_in, wot_in, cos_in, sin_in, msk_in,
                 esel_in, goff_in, ago_t, out_ext):
    ctx = ExitStack()

    consts = ctx.enter_context(tc.tile_pool(name="consts", bufs=1))
    xtp = ctx.enter_context(tc.tile_pool(name="xtp", bufs=2))
    ropep = ctx.enter_context(tc.tile_pool(name="ropep", bufs=2))
    ptp = ctx.enter_context(tc.tile_pool(name="ptp", bufs=4))
    ysp = ctx.enter_context(tc.tile_pool(name="ysp", bufs=2))
    tailp = ctx.enter_context(tc.tile_pool(name="tailp", bufs=2))
    dram = ctx.enter_context(tc.tile_pool(name="dram", bufs=1, space="DRAM"))
    # PSUM budget: proj 2 + stp 2x2 + av 2 = 8 banks
    projp = ctx.enter_context(tc.tile_pool(name="projp", bufs=2, space="PSUM"))
    pst = ctx.enter_context(tc.tile_pool(name="pst", bufs=2, space="PSUM"))
    pav = ctx.enter_context(tc.tile_pool(name="pav", bufs=1, space="PSUM"))

    # ---- constants / weights resident in SBUF ----
    ident = consts.tile([128, 128], BF16)
    make_identity(nc, ident)

    wt_sb = consts.tile([128, 8, 3 * HPC * DH], BF16)
    nc.sync.dma_start(wt_sb[:], wt_in.rearrange("(o p) m -> p o m", p=128))
    cos_sb = consts.tile([128, S], BF16)
    nc.sync.dma_start(cos_sb[:], cos_in[:])
    sin_sb = consts.tile([128, S], BF16)
    nc.sync.dma_start(sin_sb[:], sin_in[:])
    msk_sb = consts.tile([128, 2, 128], BF16)
    nc.sync.dma_start(msk_sb[:], msk_in.rearrange("p (b f) -> p b f", b=2))
    esel_sb = consts.tile([16, 8, 128], BF16)
    nc.sync.dma_start(esel_sb[:], esel_in.rearrange("p (c m) -> p c m", c=8))
    goff_sb = consts.tile([1, 1], mybir.dt.int32)
    nc.sync.dma_start(goff_sb[:], goff_in[:])

    # persistent per-core activations
    qT = consts.tile([128, S], BF16)       # [2 heads x 64dh (e|o perm), S]
    kT = consts.tile([128, S], BF16)
    vex = consts.tile([128, S // 128, 130], BF16)  # [s, chunk, v_h0 |1| v_h1 |1]
    nc.vector.memset(vex[:, :, 64:65], 1.0)
    nc.vector.memset(vex[:, :, 129:130], 1.0)

    # AllGather staging: per-window contribution [GBLK, W] bf16
    ag_in = [dram.tile([GBLK, W], BF16, name=f"ag_in{w}") for w in range(NW)]
    # Shared-HBM output, [NW * N_CORES * GBLK, W] flat (ago_t)
    WROWS = N_CORES * GBLK
    groups = [list(range(N_CORES))]

    for w in range(NW):
        ws = slice(w * W, (w + 1) * W)
        # ---- load xT window slice (host-transposed) ----
        xt_sb = xtp.tile([128, 8, W], BF16, name="xt_sb")
        nc.sync.dma_start(
            xt_sb[:], xt_in[:, ws].rearrange("(o p) m -> p o m", p=128))

        # ---- QKV projection ----
        ev_ps = projp.tile([128, W], F32, name="ev_ps", tag="pj")
        od_ps = projp.tile([128, W], F32, name="od_ps", tag="pj")
        for dst, t in ((ev_ps, 0), (od_ps, 1)):
            for d in range(8):
                nc.tensor.matmul(dst[:], wt_sb[:, d, t * 128:(t + 1) * 128],
                                 xt_sb[:, d, :], start=(d == 0), stop=(d == 7))

        # ---- RoPE (DVE), writing qT/kT 32-row strips ----
        # order frees ev_ps after two ops so the V projection can start
        cw = cos_sb[:, ws]
        sw = sin_sb[:, ws]
        t1 = ropep.tile([128, W], BF16, name="t1")
        t2 = ropep.tile([128, W], BF16, name="t2")
        t3 = ropep.tile([128, W], BF16, name="t3")
        re = ropep.tile([128, W], BF16, name="re")
        ro = ropep.tile([128, W], BF16, name="ro")
        nc.vector.tensor_tensor(t1[:], ev_ps[:], cw, mybir.AluOpType.mult)
        nc.vector.tensor_tensor(t3[:], ev_ps[:], sw, mybir.AluOpType.mult)
        nc.vector.tensor_tensor(t2[:], od_ps[:], sw, mybir.AluOpType.mult)
        nc.vector.tensor_tensor(re[:], t1[:], t2[:], mybir.AluOpType.subtract)
        nc.vector.tensor_tensor(t2[:], od_ps[:], cw, mybir.AluOpType.mult)
        nc.vector.tensor_tensor(ro[:], t3[:], t2[:], mybir.AluOpType.add)
        # qT copies on DVE (they gate this window's QK); kT on Pool (only
        # the trailing diagonal chunks need this window's keys)
        for h in range(2):
            nc.vector.tensor_copy(qT[h * 64:h * 64 + 32, ws],
                                  re[h * 32:(h + 1) * 32, :])
            nc.vector.tensor_copy(qT[h * 64 + 32:h * 64 + 64, ws],
                                  ro[h * 32:(h + 1) * 32, :])
            nc.gpsimd.tensor_copy(kT[h * 64:h * 64 + 32, ws],
                                  re[64 + h * 32:64 + (h + 1) * 32, :])
            nc.gpsimd.tensor_copy(kT[h * 64 + 32:h * 64 + 64, ws],
                                  ro[64 + h * 32:64 + (h + 1) * 32, :])

        # ---- V: project, transpose to seq-major, pack into vex ----
        v_ps = projp.tile([128, W], F32, name="v_ps", tag="pj")
        for d in range(8):
            nc.tensor.matmul(v_ps[:], wt_sb[:, d, 256:384],
                             xt_sb[:, d, :], start=(d == 0), stop=(d == 7))
        vT = ropep.tile([128, W], BF16, name="vT")
        nc.vector.tensor_copy(vT[:], v_ps[:])
        ptv = projp.tile([128, W], BF16, name="ptv", tag="pj")
        for b in range(4):
            nc.tensor.transpose(ptv[:, b * 128:(b + 1) * 128],
                                vT[:, b * 128:(b + 1) * 128], ident)
        ptv4 = ptv.rearrange("p (b d) -> p b d", b=4)
        nc.vector.tensor_copy(vex[:, w * 4:(w + 1) * 4, 0:64], ptv4[:, :, 0:64])
        nc.vector.tensor_copy(vex[:, w * 4:(w + 1) * 4, 65:129],
                              ptv4[:, :, 64:128])

        # ---- attention for window w (keys 0 .. (w+1)*512), both heads ----
        nk = 4 * (w + 1)
        av0 = pav.tile([65, W], F32, name="av0")
        av1 = pav.tile([65, W], F32, name="av1")
        avs = (av0, av1)

        pts = [None] * nk

        def emit_qk(j):
            jl = j - 4 * w
            diag = jl >= 0
            # diagonal chunk jl: columns < 128*jl are fully masked - skip
            c0 = 128 * jl if diag else 0
            stp = pst.tile([128, 2, W], F32, name="stp")
            for h in range(2):
                hs = slice(h * 64, (h + 1) * 64)
                nc.tensor.matmul(stp[:, h, c0:], kT[hs, j * 128:(j + 1) * 128],
                                 qT[hs, w * W + c0:(w + 1) * W],
                                 start=True, stop=True)
            use_dve = (not diag) and (j % 2 == 0)
            if use_dve:
                pti = ptp.tile([128, 2, W], I16, name="pti")
                nc.vector.tensor_scalar(pti[:], stp[:], EXPA, EXPB,
                                        mybir.AluOpType.mult,
                                        mybir.AluOpType.add)
                pt = pti
            else:
                pt = ptp.tile([128, 2, W], BF16, name="pt")
                nc.scalar.activation(pt[:, :, c0:], stp[:, :, c0:],
                                     mybir.ActivationFunctionType.Exp,
                                     scale=1.0 / math.sqrt(DH))
            if diag:
                # only the 128-col block on the diagonal needs the triangle
                nc.vector.tensor_tensor(pt[:, :, c0:c0 + 128],
                                        pt[:, :, c0:c0 + 128], msk_sb[:],
                                        mybir.AluOpType.mult)
            pts[j] = (pt, use_dve, c0)

        def emit_av(j):
            pt, is_i16, c0 = pts[j]
            for h in range(2):
                pth = pt[:, h, c0:]
                if is_i16:
                    pth = pth.bitcast(BF16)
                nc.tensor.matmul(avs[h][:, c0:], vex[:, j, h * 65:(h + 1) * 65],
                                 pth,
                                 start=(j == 0), stop=(j == nk - 1))
            pts[j] = None

        # software pipeline: QK_{j} runs ahead of AV_{j-1} on the PE queue
        emit_qk(0)
        for j in range(1, nk):
            emit_qk(j)
            emit_av(j - 1)
        emit_av(nk - 1)

        # ---- send unnormalized y + rowsums (Act copies, bf16) ----
        ys = ysp.tile([128, W], BF16, name="ys")
        ss = ysp.tile([64, W], BF16, name="ss")
        for h in range(2):
            nc.scalar.copy(ys[h * 64:(h + 1) * 64, :], avs[h][0:64, :])
            nc.vector.tensor_copy(ss[32 * h:32 * h + 1, :], avs[h][64:65, :])
        nc.sync.dma_start(ag_in[w][0:128, :], ys[:])
        nc.sync.dma_start(ag_in[w][128:129, :], ss[0:1, :])
        nc.sync.dma_start(ag_in[w][129:130, :], ss[32:33, :])
        nc.gpsimd.collective_compute(
            "AllGather", mybir.AluOpType.bypass, replica_groups=groups,
            ins=[ag_in[w].opt()],
            outs=[ago_t[w * WROWS:(w + 1) * WROWS, :].opt()])

    # ---- tail: read my window block (register-offset DMA), normalize,
    # output projection ----
    wot_sb = consts.tile([128, 8, D], BF16)
    nc.sync.dma_start(wot_sb[:], wot_in.rearrange("(o p) m -> p o m", p=128))

    # my window's rows start at element myrank * WROWS * W (host input)
    base = nc.sync.value_load(goff_sb[0:1, 0:1],
                              min_val=0, max_val=(NW - 1) * WROWS * W)
    blk = ago_t[0:WROWS, :].rearrange("(s p) m -> p s m", p=GBLK)
    y_src = blk[0:128]                       # [128, 8, W]
    s_src = blk[128:130]                     # [2, 8, W]
    from concourse.ap import AP as _AP
    y_dyn = _AP(y_src.tensor, base, y_src.ap)
    s_dyn = _AP(s_src.tensor, base, s_src.ap)
    yg = consts.tile([128, 8, W], BF16)
    nc.sync.dma_start(yg[:], y_dyn)
    # srows row h*8 + s = head-h rowsum of source core s
    srows = consts.tile([16, W], BF16)
    nc.sync.dma_start(srows[:].rearrange("(h s) m -> h s m", h=2), s_dyn)

    rec = tailp.tile([16, W], F32, name="rec")
    nc.vector.reciprocal(rec[:], srows[:])
    recb = tailp.tile([16, W], BF16, name="recb")
    nc.vector.tensor_copy(recb[:], rec[:])

    yn = consts.tile([128, 8, W], BF16)
    for c in range(8):
        rb = projp.tile([128, W], F32, name="rb", tag="pj")
        nc.tensor.matmul(rb[:], esel_sb[:, c, :], recb[:],
                         start=True, stop=True)
        nc.vector.tensor_tensor(yn[:, c, :], yg[:, c, :], rb[:],
                                mybir.AluOpType.mult)

    out_sb = tailp.tile([128, 4, D], F32, name="out_sb")
    for r in range(4):
        for nh in range(2):
            po = projp.tile([128, W], F32, name="po", tag="pj")
            for c in range(8):
                nc.tensor.matmul(po[:], yn[:, c, r * 128:(r + 1) * 128],
                                 wot_sb[:, c, nh * W:(nh + 1) * W],
                                 start=(c == 0), stop=(c == 7))
            nc.scalar.copy(out_sb[:, r, nh * W:(nh + 1) * W], po[:])
    nc.sync.dma_start(out_ext.rearrange("(a p) m -> p a m", p=128), out_sb[:])

    ctx.close()


_NC_CACHE = None


def _host_inputs(x, Wproj, Wo):
    """Host-side per-core input arrays (weight layout prep + tables)."""
    bf = ml_dtypes.bfloat16
    xt = np.ascontiguousarray(np.asarray(x).T).astype(bf)  # [D, S]

    invf = 1.0 / 10000.0 ** (np.arange(0, DH, 2) / DH)
    ang = np.outer(invf, np.arange(S))  # [32, S]
    cos_t = np.ascontiguousarray(np.tile(np.cos(ang), (4, 1))).astype(bf)
    sin_t = np.ascontiguousarray(np.tile(np.sin(ang), (4, 1))).astype(bf)

    p = np.arange(128)[:, None]
    f = np.arange(128)[None, :]
    msk = np.tile((p <= f).astype(np.float32), (1, 2)).astype(bf)  # [128, 256]

    wot = np.ascontiguousarray(Wo.T).astype(bf)  # [D, D]

    # esel[i, c*128 + p] = 1 if i == 8*(p >= 64) + c  (srows row = 8h + s)
    esel = np.zeros((16, 8, 128), dtype=np.float32)
    for c in range(8):
        esel[c, c, 0:64] = 1.0
        esel[8 + c, c, 64:128] = 1.0
    esel = np.ascontiguousarray(esel.reshape(16, 8 * 128)).astype(bf)

    in_maps = []
    for c in range(N_CORES):
        h0, h1 = 2 * c, 2 * c + 1
        Wq = [Wproj[64 * h:64 * h + 64, :] for h in (h0, h1)]
        Wk = [Wproj[1024 + 64 * h:1024 + 64 * h + 64, :] for h in (h0, h1)]
        Wv = [Wproj[2048 + 64 * h:2048 + 64 * h + 64, :] for h in (h0, h1)]
        evens = np.concatenate([Wq[0][::2], Wq[1][::2], Wk[0][::2], Wk[1][::2]], 0)
        odds = np.concatenate([Wq[0][1::2], Wq[1][1::2], Wk[0][1::2], Wk[1][1::2]], 0)
        vs = np.concatenate([Wv[0], Wv[1]], 0)
        wt = np.ascontiguousarray(
            np.concatenate([evens, odds, vs], 0).T).astype(bf)  # [1024, 384]

        # my window block = window c: element offset into the flat AllGather
        # output [NW * N_CORES * GBLK, W]
        goff = np.array([[c * N_CORES * GBLK * W]], dtype=np.int32)
        in_maps.append({
            "xt": xt, "wt": wt, "wot": wot,
            "cos": cos_t, "sin": sin_t, "msk": msk, "esel": esel,
            "goff": goff,
        })
    return in_maps


def kernel(x, mask, Wproj, Wo):
    global _NC_CACHE
    if _NC_CACHE is None:
        _NC_CACHE = build_graph()
    nc = _NC_CACHE
    in_maps = _host_inputs(np.asarray(x), np.asarray(Wproj), np.asarray(Wo))
    res = run_bass_kernel_spmd(nc, in_maps, core_ids=list(range(N_CORES)))
    out = np.concatenate([res.results[c]["out"] for c in range(N_CORES)], axis=0)
    return np.ascontiguousarray(out.astype(np.float32))
